# revision 55
# baseline (speedup 1.0000x reference)
"""Multi-head causal attention forward on 8 Trainium2 NeuronCores.

Reference computation (B=2, S=2048, D=1024, H=16, Dh=64):
    q/k/v = einsum("bsm,hmd->bshd", x, W_{Q,K,V}) (+ zero biases)
    scores = q @ k^T / sqrt(Dh), causal mask, softmax
    z = attn @ v
    out = einsum("bqhd,hdm->bqm", z, W_O) + sum_h b_O[h]

Sharding: core c handles batch c//4 and heads 4*(c%4) .. 4*(c%4)+3
(tensor parallel over heads x data parallel over batch). Each core
produces a partial output (sum over its 4 heads); the host sums the 4
partials per batch (the "all-reduce" of the output projection).

Device-side layout (all matmuls contract over the SBUF partition dim):
    xt  = x[b]^T                 [D=1024, S=2048]  (host pre-transposed)
    qT/kT: per head-pair packed  [128=2*Dh, S]     via lhsT=W chunks
    v:   natural [S, 4*Dh] -> per-head v' tiles [128, 65] with a ones
         column, so the z matmul also accumulates the softmax denominator
    scores^T tiles [128=k, 512=q] -> exp on ACT -> pT -> z^T accumulation
    z^T normalized by PE outer-product broadcast of 1/den
    out = z^T.T @ W_O accumulated over head pairs -> natural [S, D]
"""

import os
import sys

import numpy as np

if "/opt/trn_rl_repo" not in sys.path:
    sys.path.insert(0, "/opt/trn_rl_repo")

import concourse.bass as bass
import concourse.bacc as bacc
import concourse.tile as tile
from concourse import mybir
from concourse.bass_utils import run_bass_kernel_spmd

B, S, D, H, Dh = 2, 2048, 1024, 16, 64
HPC = 4          # heads per core
N_CORES = 8
QCH = 512        # q chunk width (one psum bank of fp32)
F32 = mybir.dt.float32
F32R = mybir.dt.float32r
BF16 = mybir.dt.bfloat16
USE_TILE_POSITION = True


def _build_masks() -> np.ndarray:
    """Two diagonal pair-tile masks, each [128, 2*QCH] laid out as (i, q).

    For the scores^T tile holding k-blocks (2t, 2t+1) of the diagonal
    region, element (kp, i*QCH + q) is valid iff rel + kp <= q where
    rel = 128*(2*dt + i) with dt in {0, 1} the diagonal tile index.
    """
    tri = (np.arange(128)[None, :] >= np.arange(128)[:, None])
    import ml_dtypes
    return np.ascontiguousarray(tri.astype(ml_dtypes.bfloat16))




def _patch_act_tables():
    """Restrict Exp/Ln membership to natural_log_exp_and_others so bacc's
    table-load pass emits one load for both (set ids keep their original
    act_info.json indices; the chosen set genuinely contains both funcs)."""
    import concourse.bacc as _bacc
    import concourse.hw_specs as _hw

    if getattr(_patch_act_tables, "_done", False):
        return
    orig = _hw.get_activation_tables

    def patched(arch):
        t = {k: set(v) for k, v in orig(arch).items()}
        combined = t.get("natural_log_exp_and_others")
        if combined:
            exp_t = mybir.ActivationFunctionType.Exp
            ln_t = next(
                (
                    getattr(mybir.ActivationFunctionType, n)
                    for n in ("Ln", "Log")
                    if hasattr(mybir.ActivationFunctionType, n)
                ),
                None,
            )
            if exp_t in combined and (ln_t is None or ln_t in combined):
                for name, s in t.items():
                    if name != "natural_log_exp_and_others":
                        s.discard(exp_t)
                        if ln_t is not None:
                            s.discard(ln_t)
        return t

    _bacc.get_activation_tables = patched
    _patch_act_tables._done = True

def build_bass() -> bass.Bass:
    _patch_act_tables()
    nc = bacc.Bacc("TRN2", target_bir_lowering=False, debug=False)

    xt_d = nc.dram_tensor("xt", [D, S], BF16, kind="ExternalInput")
    wq_d = nc.dram_tensor("wq", [2, D, 128], BF16, kind="ExternalInput")
    wk_d = nc.dram_tensor("wk", [2, D, 128], BF16, kind="ExternalInput")
    wv_d = nc.dram_tensor("wv", [D, HPC * Dh], BF16, kind="ExternalInput")
    wo_d = nc.dram_tensor("wo", [2, 128, D], BF16, kind="ExternalInput")
    out_d = nc.dram_tensor("out", [S, D], F32, kind="ExternalOutput")
    mask_d = nc.inline_tensor(_build_masks(), "cmask")

    xt = xt_d.ap()
    wq = wq_d.ap()
    wk = wk_d.ap()
    wv = wv_d.ap()
    wo = wo_d.ap()
    out = out_d.ap()
    mask = mask_d.ap()

    EXP = mybir.ActivationFunctionType.Exp
    LOG = mybir.ActivationFunctionType.Ln if hasattr(mybir.ActivationFunctionType, 'Ln') else mybir.ActivationFunctionType.Log

    with tile.TileContext(nc) as tc:
        const_pool = tc.alloc_tile_pool(name="const", bufs=1)
        persist = tc.alloc_tile_pool(name="persist", bufs=1)
        psum_s = tc.alloc_tile_pool(name="psum_s", bufs=3, space="PSUM")
        psum_z = tc.alloc_tile_pool(name="psum_z", bufs=2, space="PSUM")

        mask_sb = const_pool.tile([128, 128], BF16, name="mask_sb")
        ones32 = const_pool.tile([128, 16], F32, name="ones32")
        nc.vector.memset(ones32, 1.0)

        wo_sb = []
        for p in range(2):
            t = persist.tile([128, D], BF16, name=f"wo_sb{p}")
            wo_sb.append(t)

        qT = [persist.tile([128, S], BF16, name=f"qT{p}") for p in range(2)]
        kT = [persist.tile([128, S], BF16, name=f"kT{p}") for p in range(2)]
        vp = [persist.tile([128, 16 * 65], BF16, name=f"vp{h}") for h in range(HPC)]
        # zT[pair][qc]: [128, QCH] pair-stacked normalized z^T
        zT = [
            [persist.tile([128, QCH], BF16, name=f"zT{p}_{qc}") for qc in range(4)]
            for p in range(2)
        ]

        proj = tc.alloc_tile_pool(name="proj", bufs=1)

        # DMA order: the first projection chain needs wq pair 0 and xt chunk
        # 0 -- issue those two first so PE work starts as early as possible.
        wq_sb, wk_sb = [], []
        for p in range(2):
            wq_sb.append(proj.tile([128, 8, 128], BF16, name=f"wq_sb{p}"))
            wk_sb.append(proj.tile([128, 8, 128], BF16, name=f"wk_sb{p}"))
        wv_sb = proj.tile([128, 8, HPC * Dh], BF16, name="wv_sb")
        xt_sb = [proj.tile([128, S], BF16, name=f"xt_sb{m}") for m in range(8)]

        nc.sync.dma_start(out=wq_sb[0], in_=wq[0].rearrange("(c p) d -> p c d", p=128))
        nc.sync.dma_start(out=xt_sb[0], in_=xt[0:128, :])
        nc.sync.dma_start(out=wq_sb[1], in_=wq[1].rearrange("(c p) d -> p c d", p=128))
        nc.sync.dma_start(out=xt_sb[1], in_=xt[128:256, :])
        for p in range(2):
            nc.sync.dma_start(out=wk_sb[p], in_=wk[p].rearrange("(c p) d -> p c d", p=128))
        nc.sync.dma_start(out=wv_sb, in_=wv.rearrange("(c p) d -> p c d", p=128))
        for m in range(2, 8):
            nc.sync.dma_start(out=xt_sb[m], in_=xt[m * 128 : (m + 1) * 128, :])

        for p in range(2):
            nc.sync.dma_start(out=wo_sb[p], in_=wo[p])
        nc.sync.dma_start(out=mask_sb, in_=mask)

        # HAM warmup: ~5us of dummy matmuls with no DMA dependencies. They
        # fill the input-DMA wait at kernel start and push the PE's activity
        # monitor to the unthrottled clock before the real matmuls arrive.
        warm = proj.tile([128, QCH], BF16, name="warm")
        nc.vector.memset(warm, 1.0)
        for i in range(14):
            wps = psum_s.tile([128, 2 * QCH], F32, tag="s", name=f"wps{i}")[:, :QCH]
            nc.tensor.matmul(
                wps, lhsT=warm[:, 0:128], rhs=warm, start=True, stop=True
            )

        # ---- Phase 1: projections ----
        # q^T / k^T head-pairs: [128=2*Dh, S]
        for p in range(2):
            for w_sb, dst in ((wq_sb[p], qT[p]), (wk_sb[p], kT[p])):
                for ci in range(4):
                    ps = psum_s.tile([128, 2 * QCH], F32, tag="s", name=f"ps_qk{p}{ci}")[:, :QCH]
                    for mc in range(8):
                        nc.tensor.matmul(
                            ps,
                            lhsT=w_sb[:, mc, :],
                            rhs=xt_sb[mc][:, ci * QCH : (ci + 1) * QCH],
                            start=(mc == 0),
                            stop=(mc == 7),
                        )
                    nc.scalar.copy(dst[:, ci * QCH : (ci + 1) * QCH], ps)

        # v natural [S, 4*Dh] -> per-head v' tiles with trailing ones column.
        # The ones columns (every 65th) are written via an f32 scratch copy,
        # which rounds as the verifier requires. Only the first 4 s-blocks are
        # computed upfront (all q-chunk-0 attention needs); the rest are
        # interleaved ahead of the q-chunk that first reads them, filling
        # early-attention PE slack.
        for h in range(HPC):
            vcols = vp[h].rearrange("p (n c) -> p n c", c=65)[:, :, 64]
            nc.vector.tensor_copy(vcols, ones32)

        def emit_v(sb_lo, sb_hi):
            for sb in range(sb_lo, sb_hi):
                psv = psum_s.tile([128, 2 * QCH], F32, tag="s", name=f"ps_v{sb}")[:, : HPC * Dh]
                for mc in range(8):
                    nc.tensor.matmul(
                        psv,
                        lhsT=xt_sb[mc][:, sb * 128 : (sb + 1) * 128],
                        rhs=wv_sb[:, mc, :],
                        start=(mc == 0),
                        stop=(mc == 7),
                    )
                for h in range(HPC):
                    nc.vector.tensor_copy(
                        vp[h][:, sb * 65 : sb * 65 + 64], psv[:, h * 64 : (h + 1) * 64]
                    )

        emit_v(0, 4)

        pt_pool = tc.alloc_tile_pool(name="pt", bufs=6)
        small = tc.alloc_tile_pool(name="small", bufs=6)
        ost = tc.alloc_tile_pool(name="ost", bufs=3)

        # ---- Phase 2+3: attention, qc-major with head-pair row packing ----
        # Per k-block: the two heads of a pair compute scores concurrently on
        # the PE (tile_position row tiling, K=64 each) into one [128, 2*QCH]
        # psum tile (bank per head); one exp covers both; two z matmuls
        # accumulate z^T + denominator via the ones column of v'.
        # psz is evacuated to SBUF right away so the (slow) reciprocal and
        # normalize run off the critical path.
        for qc in range(4):
            n_kb = 4 * qc + 4
            for pair in range(2):
                psz = [
                    psum_z.tile([65, QCH], F32, tag="z", name=f"psz{pair}{qc}{hh}")
                    for hh in (0, 1)
                ]
                for kb in range(n_kb):
                    dt2 = kb - (n_kb - 4)
                    rel = max(dt2, 0) * 128
                    pss = psum_s.tile(
                        [128, 2 * QCH], F32, tag="s", name=f"pss{pair}{qc}{kb}"
                    )
                    for hh in (0, 1):
                        hoff = hh * 64
                        nc.tensor.matmul(
                            pss[:, hh * QCH + rel : (hh + 1) * QCH],
                            lhsT=kT[pair][hoff : hoff + 64, kb * 128 : (kb + 1) * 128],
                            rhs=qT[pair][
                                hoff : hoff + 64, qc * QCH + rel : (qc + 1) * QCH
                            ],
                            start=True,
                            stop=True,
                            tile_position=(hoff, 0) if USE_TILE_POSITION else None,
                        )
                    pt = pt_pool.tile(
                        [128, 2 * QCH], BF16, tag="pt", name=f"pt{pair}{qc}{kb}"
                    )
                    if rel >= 256:
                        for hh in (0, 1):
                            off = hh * QCH + rel
                            nc.scalar.activation(
                                pt[:, off : hh * QCH + QCH],
                                pss[:, off : hh * QCH + QCH],
                                EXP,
                                scale=0.125,
                            )
                    else:
                        nc.scalar.activation(pt, pss, EXP, scale=0.125)
                    # Causal handling: for a diagonal k-block at offset rel,
                    # columns q < rel are fully masked -- skipped entirely by
                    # restricting the z matmul's q-range -- and only the
                    # 128-wide triangular band needs a (cheap) mask multiply.
                    if dt2 >= 0:
                        for hh in (0, 1):
                            off = hh * QCH + rel
                            nc.vector.tensor_mul(
                                pt[:, off : off + 128], pt[:, off : off + 128], mask_sb
                            )
                    for hh in (0, 1):
                        nc.tensor.matmul(
                            psz[hh][:, rel:QCH],
                            lhsT=vp[2 * pair + hh][:, kb * 65 : (kb + 1) * 65],
                            rhs=pt[:, hh * QCH + rel : (hh + 1) * QCH],
                            start=(kb == 0),
                            stop=(kb == n_kb - 1),
                        )
                # Evacuate both psz tiles to SBUF first (on ScalarE, whose
                # queue is short) so the PSUM slots free before the slow
                # reciprocal chain runs; the next pair's z-accumulation is
                # gated on these slots.
                zs2 = []
                for hh in (0, 1):
                    zs = small.tile([65, QCH], F32, tag="zs", name=f"zs{pair}{qc}{hh}")
                    nc.vector.tensor_copy(zs, psz[hh])
                    zs2.append(zs)
                for hh in (0, 1):
                    zs = zs2[hh]
                    rden = small.tile([1, QCH], F32, tag="rden", name=f"rd{pair}{qc}{hh}")
                    nc.vector.reciprocal(rden, zs[64:65, :])
                    rb = small.tile([64, QCH], F32, tag="rb", name=f"rb{pair}{qc}{hh}")
                    nc.gpsimd.partition_broadcast(rb, rden)
                    nc.vector.tensor_mul(
                        zT[pair][qc][hh * 64 : (hh + 1) * 64, :], zs[0:64, :], rb
                    )
            for qi in range(4):
                qb = 4 * qc + qi
                outsb = ost.tile([128, D], F32, tag="ost", name=f"outsb{qb}")
                for mc in range(2):
                    pso = psum_s.tile([128, 2 * QCH], F32, tag="s", name=f"pso{qb}{mc}")[:, :QCH]
                    for p in range(2):
                        nc.tensor.matmul(
                            pso,
                            lhsT=zT[p][qc][:, qi * 128 : (qi + 1) * 128],
                            rhs=wo_sb[p][:, mc * QCH : (mc + 1) * QCH],
                            start=(p == 0),
                            stop=(p == 1),
                        )
                    nc.vector.tensor_copy(outsb[:, mc * QCH : (mc + 1) * QCH], pso)
                nc.sync.dma_start(out=out[qb * 128 : (qb + 1) * 128, :], in_=outsb)

        ost.release()
        small.release()
        pt_pool.release()
        proj.release()
        psum_z.release()
        psum_s.release()
        persist.release()
        const_pool.release()

    nc.compile()
    return nc


_NC_CACHE: list = []


def _get_nc() -> bass.Bass:
    if not _NC_CACHE:
        _NC_CACHE.append(build_bass())
    return _NC_CACHE[0]


def _core_inputs(x, W_Q, W_K, W_V, W_O, c):
    b = c // HPC
    h0 = HPC * (c % HPC)
    wq = np.stack(
        [W_Q[h0 + 2 * p : h0 + 2 * p + 2].transpose(1, 0, 2).reshape(D, 128) for p in range(2)]
    )
    wk = np.stack(
        [W_K[h0 + 2 * p : h0 + 2 * p + 2].transpose(1, 0, 2).reshape(D, 128) for p in range(2)]
    )
    wv = W_V[h0 : h0 + HPC].transpose(1, 0, 2).reshape(D, HPC * Dh)
    wo = np.stack([W_O[h0 + 2 * p : h0 + 2 * p + 2].reshape(128, D) for p in range(2)])
    import ml_dtypes

    bf = ml_dtypes.bfloat16
    return {
        "xt": np.ascontiguousarray(x[b].T.astype(bf)),
        "wq": np.ascontiguousarray(wq.astype(bf)),
        "wk": np.ascontiguousarray(wk.astype(bf)),
        "wv": np.ascontiguousarray(wv.astype(bf)),
        "wo": np.ascontiguousarray(wo.astype(bf)),
    }


def _ensure_ntff_hook():
    """Install the axon NTFF profile hook if the image's antenv lacks it.

    Only needed for trace=True runs (test harness); the grading path
    (kernel()) never calls this.
    """
    try:
        from antenv.axon_hooks import get_axon_ntff_profile_hook  # noqa: F401
        return
    except ImportError:
        pass
    import types

    import antenv

    holder = {"hook": None}
    mod = types.ModuleType("antenv.axon_hooks")
    mod.set_axon_ntff_profile_hook = lambda h: holder.__setitem__("hook", h)
    mod.get_axon_ntff_profile_hook = lambda: holder["hook"]
    sys.modules["antenv.axon_hooks"] = mod
    antenv.axon_hooks = mod
    try:
        if "/root/.axon_site" not in sys.path:
            sys.path.insert(0, "/root/.axon_site")
        from trn_agent_boot.trn_boot import _ntff_profile_via_ctypes

        so = "/opt/axon/libaxon_pjrt.so"
        if os.path.exists(so):
            mod.set_axon_ntff_profile_hook(_ntff_profile_via_ctypes(so))
    except Exception as e:  # degrade to no tracing
        print(f"NTFF hook install failed: {e}", file=sys.stderr)
    # artifact upload needs S3 creds this container may not have
    import concourse.bass_utils as bu

    bu.upload_artifacts = lambda tmpdir: f"local://{tmpdir}"


def _run(inputs: dict, trace: bool = False):
    x = np.asarray(inputs["x"], np.float32)
    W_Q = np.asarray(inputs["W_Q"], np.float32)
    W_K = np.asarray(inputs["W_K"], np.float32)
    W_V = np.asarray(inputs["W_V"], np.float32)
    W_O = np.asarray(inputs["W_O"], np.float32)
    b_O = np.asarray(inputs["b_O"], np.float32)

    if trace:
        _ensure_ntff_hook()
    nc = _get_nc()
    in_maps = [_core_inputs(x, W_Q, W_K, W_V, W_O, c) for c in range(N_CORES)]
    res = run_bass_kernel_spmd(nc, in_maps, core_ids=list(range(N_CORES)), trace=trace)

    out = np.zeros((B, S, D), np.float32)
    for c in range(N_CORES):
        out[c // HPC] += res.results[c]["out"]
    out += b_O.sum(axis=0)  # b_O is [H, D]; reference adds sum over heads
    return out, res


def kernel(**inputs) -> np.ndarray:
    # b_Q/b_K/b_V are zero in the reference's setup_inputs; the device
    # kernel folds them out. Guard with an exact fallback just in case.
    for name in ("b_Q", "b_K", "b_V"):
        if name in inputs and np.any(np.asarray(inputs[name])):
            return _kernel_numpy_fallback(**inputs)
    out, _ = _run(inputs)
    return out


def _kernel_numpy_fallback(x, W_Q, b_Q, W_K, b_K, W_V, W_O, b_V, b_O):
    x = np.asarray(x, np.float32)
    q = np.einsum("bqm,hmd->bqhd", x, W_Q) + b_Q
    k = np.einsum("bkm,hmd->bkhd", x, W_K) + b_K
    v = np.einsum("bkm,hmd->bkhd", x, W_V) + b_V
    s = np.einsum("bqhd,bkhd->bhqk", q, k) / np.sqrt(np.float32(W_Q.shape[-1]))
    causal = np.tril(np.ones((x.shape[1], x.shape[1]), bool))
    s = np.where(causal, s, np.float32(-1e9))
    s = s - s.max(-1, keepdims=True)
    e = np.exp(s)
    attn = e / e.sum(-1, keepdims=True)
    z = np.einsum("bhqk,bkhd->bqhd", attn, v)
    return np.einsum("bqhd,hdm->bqm", z, W_O) + b_O.sum(0)
